# revision 36
# baseline (speedup 1.0000x reference)
"""Trainium2 Bass kernel for nn_Attention_63745904608049.

Relative-position attention (TransformerXL-style shift, Enformer-style pos
basis), batch 1, seq 2048, d_model 1536, 8 heads x 64. Head-parallel over 8
NeuronCores; the positional-score relative shift is realized as a DRAM
roundtrip (straight write, sheared flat-AP read).

v2: positional table P = emb @ sum_h(Wp) + b precomputed on host (weight
preprocessing; emb is an input-independent constant), removing the first
AllGather; warmup barrier collective issued at graph start; scores/softmax/
attn*V/output-AllGather/final-projection all pipelined per 128-row query
tile with the row-sum of exp-scores computed for free via a ones-column in
the V tiles; output AllGather split into 4 chunks overlapped with compute.

Self-contained: hardcodes shapes, builds one SPMD Bass graph, runs it on
cores 0-7 via run_bass_kernel_spmd, and reassembles the full output.
"""
import contextlib
import ctypes
import math
import os
import sys
import types

import numpy as np
import ml_dtypes

import concourse.bass as bass
import concourse.mybir as mybir
from concourse.tile import TileContext
from concourse.masks import make_identity
from concourse.bass_utils import run_bass_kernel_spmd

# ----------------------------------------------------------------------------
# problem constants
N = 2048
DM = 1536
H = 8
HD = 64
INNER = H * HD            # 512
NCORES = 8
QT = N // 128             # 16 query tiles
WIN = 2175                # per-q-tile pos table window (128 + 2048 - 1)
TSTRIDE = 2304            # padded row stride of the T scratch (elements)
CHUNKS = [(0, 512), (512, 512), (1024, 512), (1536, 512), (2048, 127)]
F32 = mybir.dt.float32
BF16 = mybir.dt.bfloat16
FP16 = mybir.dt.float16

_LAST_RESULT = None       # BassKernelResults of the last run (for test.py)


# ----------------------------------------------------------------------------
# axon NTFF profiling hook (lets BASS_TRACE=1 produce exec_time_ns under axon)
def _install_ntff_hook(so_path="/opt/axon/libaxon_pjrt.so"):
    try:
        import antenv.axon_hooks  # noqa: F401
        return
    except ImportError:
        pass
    try:
        lib = ctypes.CDLL(so_path)
    except OSError:
        return
    if not hasattr(lib, "axon_start_nrt_profile"):
        return
    lib.axon_start_nrt_profile.argtypes = [ctypes.POINTER(ctypes.c_int64), ctypes.c_size_t]
    lib.axon_start_nrt_profile.restype = ctypes.c_int64
    lib.axon_stop_nrt_profile.argtypes = [ctypes.c_char_p]
    lib.axon_stop_nrt_profile.restype = ctypes.c_int64

    @contextlib.contextmanager
    def _hook(output_dir, device_ids):
        import jax
        jax.devices()
        if device_ids:
            ids = (ctypes.c_int64 * len(device_ids))(*device_ids)
            rc = lib.axon_start_nrt_profile(ids, len(device_ids))
        else:
            rc = lib.axon_start_nrt_profile(None, 0)
        if rc != 0:
            raise RuntimeError(f"axon_start_nrt_profile rc={rc}")
        try:
            yield
        finally:
            n = lib.axon_stop_nrt_profile(str(output_dir).encode())
            print(f"ntff profile: {n} file(s) written to {output_dir}")

    mod = types.ModuleType("antenv.axon_hooks")
    mod.get_axon_ntff_profile_hook = lambda: _hook
    mod.set_axon_ntff_profile_hook = lambda h: None
    sys.modules["antenv.axon_hooks"] = mod


_install_ntff_hook()


# ----------------------------------------------------------------------------
# BIR post-processing: this container's walrus build rejects instructions with
# more than one sync wait; split extra waits onto preceding NoOps.
def _split_waits(bir_bytes, maxw=1):
    import json
    d = json.loads(bir_bytes)
    counter = [0]
    for fn in d["functions"]:
        for blk in fn["blocks"]:
            out = []
            for ins in blk["instructions"]:
                si = ins.get("sync_info")
                waits = (si or {}).get("on_wait") or []
                if len(waits) > maxw:
                    excess = waits[:-maxw]
                    ins["sync_info"]["on_wait"] = waits[-maxw:]
                    for i in range(0, len(excess), maxw):
                        counter[0] += 1
                        nop = {
                            "engine": ins["engine"],
                            "ins": [],
                            "outs": [],
                            "name": f"I-waitsplit-{counter[0]}",
                            "opcode": "NoOp",
                            "sync_info": {"on_update": [],
                                          "on_wait": excess[i:i + maxw]},
                        }
                        if "debug" in ins:
                            nop["debug"] = ins["debug"]
                        out.append(nop)
                out.append(ins)
            blk["instructions"] = out
    return json.dumps(d).encode()


# ----------------------------------------------------------------------------
# host-side positional embedding table (pure function of N, DM)
_POS_CACHE = {}


def _pos_embed():
    if "emb" in _POS_CACHE:
        return _POS_CACHE["emb"]
    n, fs = N, DM
    nb = fs // 6  # 256
    dist = np.arange(-n + 1, n, dtype=np.float64)
    adist = np.abs(dist)[:, None]

    max_range = math.log(n) / math.log(2.0)
    half_life = 2.0 ** np.linspace(3.0, max_range, nb)
    exp_feat = np.exp(-math.log(2.0) / half_life[None, :] * adist)

    with np.errstate(over="ignore"):
        center_widths = 2.0 ** np.arange(1, nb + 1, dtype=np.float64) - 1.0
    cmask_feat = (center_widths[None, :] > adist).astype(np.float64)

    stddev = n / (2.0 * nb)
    start_mean = n / nb
    mean = np.linspace(start_mean, float(n), nb)[None, :]
    conc = (mean / stddev) ** 2
    rate = mean / stddev ** 2
    with np.errstate(divide="ignore", invalid="ignore"):
        log_unnorm = (conc - 1.0) * np.log(adist) - rate * adist
    lgamma = np.vectorize(math.lgamma)
    log_norm = lgamma(conc) - conc * np.log(rate)
    with np.errstate(invalid="ignore"):
        prob = np.exp(log_unnorm - log_norm) + 1e-08
    prob = np.nan_to_num(prob, nan=1e-08)  # adist=0: 0*inf -> use limit 0, then +eps
    # recompute the adist == 0 row exactly: log_unnorm = -inf -> exp -> 0
    zrow = np.where(adist[:, 0] == 0)[0]
    prob[zrow, :] = 1e-08
    gamma_feat = prob / prob.max(axis=-1, keepdims=True)

    emb = np.concatenate([exp_feat, cmask_feat, gamma_feat], axis=-1)
    emb = np.concatenate([emb, np.sign(dist)[:, None] * emb], axis=-1)
    _POS_CACHE["emb"] = emb  # (4095, 1536) float64
    return emb


# ----------------------------------------------------------------------------
# device graph (identical for all cores; per-core data differs)
_GRAPH_CACHE = {}


def _build_graph():
    if "nc" in _GRAPH_CACHE:
        return _GRAPH_CACHE["nc"]
    debug = bool(os.environ.get("KERNEL_DEBUG"))
    nc = bass.Bass()

    xT = nc.declare_dram_parameter("xT", [DM, N], FP16, isOutput=False)
    wq = nc.declare_dram_parameter("wq", [DM, HD], FP16, isOutput=False)
    wkv = nc.declare_dram_parameter("wkv", [DM, 2 * HD], FP16, isOutput=False)
    ptab = nc.declare_dram_parameter("ptab", [HD, 2 * N], FP16, isOutput=False)
    cbias = nc.declare_dram_parameter("cbias", [HD, 1], F32, isOutput=False)
    pbias = nc.declare_dram_parameter("pbias", [HD, 1], F32, isOutput=False)
    wout = nc.declare_dram_parameter("wout", [INNER, 192], BF16, isOutput=False)
    bout = nc.declare_dram_parameter("bout", [1, 192], F32, isOutput=False)
    out_ext = nc.declare_dram_parameter("out", [N, 192], F32, isOutput=True)
    if debug:
        dbg_an = nc.declare_dram_parameter("dbg_an", [256, N], BF16, isOutput=True)
        dbg_og = nc.declare_dram_parameter("dbg_og", [HD, N], BF16, isOutput=True)
        dbg_sh = nc.declare_dram_parameter("dbg_sh", [256, N], FP16, isOutput=True)
        dbg_at = nc.declare_dram_parameter("dbg_at", [128, 2 * N], BF16, isOutput=True)
        dbg_rs = nc.declare_dram_parameter("dbg_rs", [HD, N], F32, isOutput=True)
    debug3 = bool(os.environ.get("KERNEL_DEBUG3"))
    if debug3:
        dbg3_og = nc.declare_dram_parameter("dbg3_og", [HD, 512], BF16, isOutput=True)
        dbg3_rs = nc.declare_dram_parameter("dbg3_rs", [HD, 512], F32, isOutput=True)
        dbg3_at = nc.declare_dram_parameter("dbg3_at", [128, 2 * N], BF16, isOutput=True)
        dbg3_an = nc.declare_dram_parameter("dbg3_an", [256, N], BF16, isOutput=True)
        dbg3_of = nc.declare_dram_parameter("dbg3_of", [512, 512], BF16, isOutput=True)

    # internal DRAM
    t_dram = [nc.dram_tensor(f"tscratch{a}", [128, TSTRIDE], FP16) for a in range(QT)]
    rs_dram = [nc.dram_tensor(f"rsrow{g}", [1, 512], F32) for g in range(4)]
    warm_in = nc.dram_tensor("warm_in", [1, 16], F32)
    warm_out = nc.dram_tensor("warm_out", [NCORES, 16], F32, addr_space="Shared")
    oag_in = [nc.dram_tensor(f"oag_in{g}", [HD, 512], BF16) for g in range(4)]
    oag_out = [nc.dram_tensor(f"oag_out{g}", [NCORES * HD, 512], BF16,
                              addr_space="Shared") for g in range(4)]

    groups = [list(range(NCORES))]
    Act = mybir.ActivationFunctionType

    with TileContext(nc) as tc:
        with contextlib.ExitStack() as ctx:
            persist = ctx.enter_context(tc.tile_pool(name="persist", bufs=1))
            work = ctx.enter_context(tc.tile_pool(name="work", bufs=2))

            # ------------- warmup collective: absorb bootstrap barrier ------
            nc.gpsimd.collective_compute(
                "AllGather", mybir.AluOpType.bypass, replica_groups=groups,
                ins=[warm_in.ap().opt()], outs=[warm_out.ap().opt()])

            # ---------------- phase 1: projections ----------------
            qcT = persist.tile([HD, N], FP16, tag="qcT")
            qpT = persist.tile([HD, N], FP16, tag="qpT")
            kvT = persist.tile([128, N], FP16, tag="kvT")   # k rows 0:64, v rows 64:128
            PT = persist.tile([HD, 2 * N], FP16, tag="PT")
            # vsb[k]: [seq 128, hd 64 | ones 1] -> row sums for free in attn*V
            vsb = [persist.tile([128, HD + 1], BF16, tag=f"v{k}", name=f"v{k}")
                   for k in range(QT)]
            ident = persist.tile([128, 128], FP16, tag="ident")
            make_identity(nc, ident)

            cb_sb = persist.tile([HD, 1], F32, tag="cb")
            pb_sb = persist.tile([HD, 1], F32, tag="pb")
            nc.sync.dma_start(out=cb_sb, in_=cbias[:, :])
            nc.sync.dma_start(out=pb_sb, in_=pbias[:, :])
            nc.gpsimd.dma_start(out=PT, in_=ptab[:, :])

            wout_sb = []
            for r in range(4):
                t = persist.tile([128, 192], BF16, tag=f"wo{r}", name=f"wo{r}")
                nc.gpsimd.dma_start(out=t, in_=wout[128 * r:128 * (r + 1), :])
                wout_sb.append(t)
            bout_sb = persist.tile([128, 192], F32, tag="bo")
            bout_bc = bass.AP(tensor=bout.ap().tensor, offset=0,
                              ap=[[0, 128], [1, 192]])
            nc.gpsimd.dma_start(out=bout_sb, in_=bout_bc)

            with contextlib.ExitStack() as ph1:
                wpool = ph1.enter_context(tc.tile_pool(name="wpool", bufs=1))
                xstream = ph1.enter_context(tc.tile_pool(name="xstream", bufs=2))
                ppsum = ph1.enter_context(
                    tc.tile_pool(name="ppsum", bufs=1, space="PSUM"))

                wq_sb, wkv_sb = [], []
                for f in range(12):
                    t = wpool.tile([128, HD], FP16, tag=f"wq{f}", name=f"wq{f}")
                    eng = nc.sync if f % 2 == 0 else nc.scalar
                    eng.dma_start(out=t, in_=wq[128 * f:128 * (f + 1), :])
                    wq_sb.append(t)
                    t = wpool.tile([128, 2 * HD], FP16, tag=f"wkv{f}", name=f"wkv{f}")
                    eng = nc.scalar if f % 2 == 0 else nc.sync
                    eng.dma_start(out=t, in_=wkv[128 * f:128 * (f + 1), :])
                    wkv_sb.append(t)

                # single-pass projections: 8 accumulating psum banks
                q_ps = [ppsum.tile([HD, 512], F32, tag="pq", bufs=4,
                                   name=f"qps{i}") for i in range(4)]
                kv_ps = [ppsum.tile([128, 512], F32, tag="pkv", bufs=4,
                                    name=f"kvps{i}") for i in range(4)]
                for f in range(12):
                    xt = xstream.tile([128, N], FP16, tag="xt")
                    eng = nc.sync if f % 2 == 0 else nc.scalar
                    eng.dma_start(out=xt, in_=xT[128 * f:128 * (f + 1), :])
                    for i in range(4):
                        nc.tensor.matmul(q_ps[i], wq_sb[f],
                                         xt[:, 512 * i:512 * (i + 1)],
                                         start=(f == 0), stop=(f == 11))
                    for i in range(4):
                        nc.tensor.matmul(kv_ps[i], wkv_sb[f],
                                         xt[:, 512 * i:512 * (i + 1)],
                                         start=(f == 0), stop=(f == 11))
                # copy-out: qpT first (unblocks T-matmul pipeline), then qcT/kv
                for i in range(4):
                    nc.scalar.activation(qpT[:, 512 * i:512 * (i + 1)], q_ps[i],
                                         Act.Identity, bias=pb_sb)
                for i in range(4):
                    nc.scalar.activation(qcT[:, 512 * i:512 * (i + 1)], q_ps[i],
                                         Act.Identity, bias=cb_sb)
                    nc.vector.tensor_copy(kvT[:, 512 * i:512 * (i + 1)], kv_ps[i])
                # ones column helper: f(0*x + 1) = 1 via ACT immediate
                onescol = persist.tile([128, 1], BF16, tag="ones")
                nc.scalar.activation(onescol, bout_sb[:, 0:1],
                                     Act.Identity, scale=0.0, bias=1.0)
                # v transpose to seq-major bf16 (+ ones column for row sums)
                for k in range(QT):
                    tp = ppsum.tile([128, HD], FP16, tag="pq", bufs=4)
                    nc.tensor.transpose(tp, kvT[HD:128, 128 * k:128 * (k + 1)],
                                        ident[HD:128, HD:128])
                    nc.vector.tensor_copy(vsb[k][:, 0:HD], tp)
                    nc.vector.tensor_copy(vsb[k][:, HD:HD + 1], onescol)

            # ---------------- phase 2: pipelined scores/softmax/av/AG/fin ---
            psum = ctx.enter_context(tc.tile_pool(name="psum", bufs=1, space="PSUM"))

            def produce(a):
                """T matmuls -> fp16 tsb -> DRAM write -> sheared read."""
                w0 = 1920 - 128 * a
                tsb = work.tile([128, WIN], FP16, tag="tsb", bufs=3,
                                name=f"tsb{a}")
                for ci, (off, w) in enumerate(CHUNKS):
                    tp = psum.tile([128, 512], F32, tag="T", bufs=2,
                                   name=f"tp{a}_{off}")
                    nc.tensor.matmul(tp[:, :w], qpT[:, 128 * a:128 * (a + 1)],
                                     PT[:, w0 + off:w0 + off + w],
                                     start=True, stop=True)
                    if ci in (1, 3):
                        nc.vector.tensor_copy(tsb[:, off:off + w], tp[:, :w])
                    else:
                        nc.scalar.copy(tsb[:, off:off + w], tp[:, :w])
                nc.gpsimd.dma_start(out=t_dram[a][:, 0:WIN], in_=tsb[:, 0:WIN])
                shear = work.tile([128, N], FP16, tag="shear", bufs=3,
                                  name=f"shear{a}")
                src = bass.AP(tensor=t_dram[a].ap().tensor, offset=127,
                              ap=[[TSTRIDE - 1, 128], [1, N]])
                nc.scalar.dma_start(out=shear, in_=src)
                return shear

            an = [persist.tile([128, N], BF16, tag=f"an{a}", name=f"an{a}")
                  for a in range(QT)]

            def consume(a, shear):
                """scores = qk (psum) + shear (DVE), exp (ACT) -> an[a]."""
                for j in range(4):
                    sp = psum.tile([128, 512], F32, tag="sc", bufs=3,
                                   name=f"sp{a}_{j}")
                    nc.tensor.matmul(sp, qcT[:, 128 * a:128 * (a + 1)],
                                     kvT[0:HD, 512 * j:512 * (j + 1)],
                                     start=True, stop=True)
                    nc.vector.tensor_add(sp, sp,
                                         shear[:, 512 * j:512 * (j + 1)])
                    nc.scalar.activation(an[a][:, 512 * j:512 * (j + 1)], sp,
                                         Act.Exp)
                if debug and a in (0, 6):
                    r = 0 if a == 0 else 128
                    nc.gpsimd.dma_start(out=dbg_an[r:r + 128, :], in_=an[a])
                    nc.gpsimd.dma_start(out=dbg_sh[r:r + 128, :], in_=shear)

            def at3(a):
                g, b = a // 4, a % 4
                if b == 0:
                    atG[g] = work.tile([128, 4, QT, 128], BF16, tag="atG",
                                       bufs=4, name=f"atG{g}")
                nc.sync.dma_start_transpose(atG[g][:, b, :, :], an[a])

            def av_group(g):
                """attn*V for 4 q-tiles; row 64 of otp = row sums (ones col)."""
                otp = psum.tile([HD + 1, 512], F32, tag="av", bufs=2,
                                name=f"otp{g}")
                for k in range(QT):
                    nc.tensor.matmul(otp, vsb[k], atG[g][:, :, k, :],
                                     start=(k == 0), stop=(k == QT - 1))
                rsr = work.tile([1, 512], F32, tag="rsr", bufs=2, name=f"rsr{g}")
                nc.vector.reciprocal(rsr, otp[HD:HD + 1, :])
                nc.sync.dma_start(out=rs_dram[g][:, :], in_=rsr)
                rsb = work.tile([HD, 512], F32, tag="rsb", bufs=1, name=f"rsb{g}")
                rs_bc = bass.AP(tensor=rs_dram[g].ap().tensor, offset=0,
                                ap=[[0, HD], [1, 512]])
                nc.scalar.dma_start(out=rsb, in_=rs_bc)
                og = work.tile([HD, 512], BF16, tag="og", bufs=4, name=f"og{g}")
                nc.vector.tensor_mul(og, otp[0:HD, :], rsb)
                if debug3 and g == 3:
                    dbg3_refs["og"] = og
                    dbg3_refs["rsb"] = rsb
                if debug:
                    nc.gpsimd.dma_start(out=dbg_og[:, 512 * g:512 * (g + 1)],
                                        in_=og)
                    nc.gpsimd.dma_start(out=dbg_rs[:, 512 * g:512 * (g + 1)],
                                        in_=rsb)
                    if g == 1:
                        nc.gpsimd.dma_start(out=dbg_at[:, 0:N],
                                            in_=atG[g][:, 2, :, :])
                        nc.gpsimd.dma_start(out=dbg_at[:, N:2 * N],
                                            in_=atG[g][:, 3, :, :])
                nc.scalar.dma_start(out=oag_in[g][:, :], in_=og)
                nc.gpsimd.collective_compute(
                    "AllGather", mybir.AluOpType.bypass, replica_groups=groups,
                    ins=[oag_in[g].ap().opt()], outs=[oag_out[g].ap().opt()])

            def fin(g):
                ofull = []
                for r in range(4):
                    t = work.tile([128, 512], BF16, tag="of", bufs=4,
                                  name=f"of{g}{r}")
                    eng = nc.sync if r % 2 == 0 else nc.scalar
                    eng.dma_start(out=t, in_=oag_out[g][128 * r:128 * (r + 1), :])
                    ofull.append(t)
                if debug3 and g == 3:
                    dbg3_refs["of"] = ofull
                for b in range(4):
                    fp = psum.tile([128, 192], F32, tag="fin", bufs=1,
                                   name=f"fp{g}_{b}")
                    for r in range(4):
                        nc.tensor.matmul(fp, ofull[r][:, 128 * b:128 * (b + 1)],
                                         wout_sb[r], start=(r == 0), stop=(r == 3))
                    ob = work.tile([128, 192], F32, tag="ob", bufs=4,
                                   name=f"ob{g}_{b}")
                    nc.vector.tensor_add(ob, fp, bout_sb)
                    m = 4 * g + b
                    eng = nc.sync if b % 2 == 0 else nc.scalar
                    eng.dma_start(out=out_ext[128 * m:128 * (m + 1), :], in_=ob)

            atG = {}
            dbg3_refs = {}

            LOOK = 3
            shears = {}
            for a in range(LOOK):
                shears[a] = produce(a)
            for a in range(QT):
                if a + LOOK < QT:
                    shears[a + LOOK] = produce(a + LOOK)
                consume(a, shears.pop(a))
                at3(a)
                if a % 4 == 3:
                    av_group(a // 4)
                if a == 9:
                    fin(0)
                if a == 12:
                    fin(1)
            fin(2)
            fin(3)
            if debug3:
                nc.gpsimd.dma_start(out=dbg3_og[:, :], in_=dbg3_refs["og"])
                nc.gpsimd.dma_start(out=dbg3_rs[:, :], in_=dbg3_refs["rsb"])
                nc.gpsimd.dma_start(out=dbg3_at[:, 0:N], in_=atG[3][:, 2, :, :])
                nc.gpsimd.dma_start(out=dbg3_at[:, N:2 * N],
                                    in_=atG[3][:, 3, :, :])
                nc.gpsimd.dma_start(out=dbg3_an[0:128, :], in_=an[14])
                nc.gpsimd.dma_start(out=dbg3_an[128:256, :], in_=an[15])
                for r in range(4):
                    nc.gpsimd.dma_start(out=dbg3_of[128 * r:128 * (r + 1), :],
                                        in_=dbg3_refs["of"][r])

    # wait-split post-processing hook
    orig = nc.to_json_bytes
    nc.to_json_bytes = lambda: _split_waits(orig())
    _GRAPH_CACHE["nc"] = nc
    return nc


# ----------------------------------------------------------------------------
def _prep_inputs(x, Wq, Wk, Wv, content_bias, pos_bias, Wp_w, Wp_b, Wout_w, Wout_b):
    x = np.ascontiguousarray(np.asarray(x, dtype=np.float32))
    Wq = np.asarray(Wq, np.float32); Wk = np.asarray(Wk, np.float32)
    Wv = np.asarray(Wv, np.float32)
    content_bias = np.asarray(content_bias, np.float32)
    pos_bias = np.asarray(pos_bias, np.float32)
    Wp_w = np.asarray(Wp_w, np.float32); Wp_b = np.asarray(Wp_b, np.float32)
    Wout_w = np.asarray(Wout_w, np.float32); Wout_b = np.asarray(Wout_b, np.float32)

    scale = HD ** -0.5
    xT = np.ascontiguousarray(x[0].T)                    # (1536, 2048)
    emb = _pos_embed()                                   # (4095, 1536) f64
    wp_sum = Wp_w.reshape(DM, H, HD).sum(axis=1)         # (1536, 64)
    wp_b_sum = Wp_b.reshape(H, HD).sum(axis=0)           # (64,)
    # positional table: P = emb @ wp_sum + b  (weight preprocessing)
    P = emb @ wp_sum.astype(np.float64) + wp_b_sum.astype(np.float64)
    PTh = np.zeros((HD, 2 * N), np.float16)
    PTh[:, :2 * N - 1] = P.T.astype(np.float16)          # (64, 4096)
    xT16 = xT.astype(np.float16)

    in_maps = []
    for c in range(NCORES):
        sl = slice(HD * c, HD * (c + 1))
        in_maps.append({
            "xT": xT16,
            "wq": np.ascontiguousarray(Wq[:, sl] * scale).astype(np.float16),
            "wkv": np.ascontiguousarray(
                np.concatenate([Wk[:, sl], Wv[:, sl]], axis=1)).astype(np.float16),
            "ptab": PTh,
            "cbias": np.ascontiguousarray(content_bias[c, 0, :, None]),
            "pbias": np.ascontiguousarray(pos_bias[c, 0, :, None]),
            "wout": np.ascontiguousarray(
                Wout_w[:, 192 * c:192 * (c + 1)]).astype(ml_dtypes.bfloat16),
            "bout": np.ascontiguousarray(Wout_b[None, 192 * c:192 * (c + 1)]),
        })
    return in_maps


def kernel(x, Wq, Wk, Wv, content_bias, pos_bias, Wp_w, Wp_b, Wout_w, Wout_b):
    global _LAST_RESULT
    in_maps = _prep_inputs(x, Wq, Wk, Wv, content_bias, pos_bias,
                           Wp_w, Wp_b, Wout_w, Wout_b)
    nc = _build_graph()
    trace = bool(os.environ.get("KERNEL_TRACE"))
    res = run_bass_kernel_spmd(nc, in_maps, core_ids=list(range(NCORES)),
                               trace=trace, trace_cores=[0] if trace else None)
    _LAST_RESULT = res
    out = np.concatenate([res.results[c]["out"] for c in range(NCORES)], axis=1)
    return out[None].astype(np.float32)


# revision 38
# speedup vs baseline: 1.0138x; 1.0138x over previous
"""Trainium2 Bass kernel for nn_Attention_63745904608049.

Relative-position attention (TransformerXL-style shift, Enformer-style pos
basis), batch 1, seq 2048, d_model 1536, 8 heads x 64. Head-parallel over 8
NeuronCores; the positional-score relative shift is realized as a DRAM
roundtrip (straight write, sheared flat-AP read).

v2: positional table P = emb @ sum_h(Wp) + b precomputed on host (weight
preprocessing; emb is an input-independent constant), removing the first
AllGather; warmup barrier collective issued at graph start; scores/softmax/
attn*V/output-AllGather/final-projection all pipelined per 128-row query
tile with the row-sum of exp-scores computed for free via a ones-column in
the V tiles; output AllGather split into 4 chunks overlapped with compute.

Self-contained: hardcodes shapes, builds one SPMD Bass graph, runs it on
cores 0-7 via run_bass_kernel_spmd, and reassembles the full output.
"""
import contextlib
import ctypes
import math
import os
import sys
import types

import numpy as np
import ml_dtypes

import concourse.bass as bass
import concourse.mybir as mybir
from concourse.tile import TileContext
from concourse.masks import make_identity
from concourse.bass_utils import run_bass_kernel_spmd

# ----------------------------------------------------------------------------
# problem constants
N = 2048
DM = 1536
H = 8
HD = 64
INNER = H * HD            # 512
NCORES = 8
QT = N // 128             # 16 query tiles
WIN = 2175                # per-q-tile pos table window (128 + 2048 - 1)
TSTRIDE = 2304            # padded row stride of the T scratch (elements)
CHUNKS = [(0, 512), (512, 512), (1024, 512), (1536, 512), (2048, 127)]
F32 = mybir.dt.float32
BF16 = mybir.dt.bfloat16
FP16 = mybir.dt.float16

_LAST_RESULT = None       # BassKernelResults of the last run (for test.py)


# ----------------------------------------------------------------------------
# axon NTFF profiling hook (lets BASS_TRACE=1 produce exec_time_ns under axon)
def _install_ntff_hook(so_path="/opt/axon/libaxon_pjrt.so"):
    try:
        import antenv.axon_hooks  # noqa: F401
        return
    except ImportError:
        pass
    try:
        lib = ctypes.CDLL(so_path)
    except OSError:
        return
    if not hasattr(lib, "axon_start_nrt_profile"):
        return
    lib.axon_start_nrt_profile.argtypes = [ctypes.POINTER(ctypes.c_int64), ctypes.c_size_t]
    lib.axon_start_nrt_profile.restype = ctypes.c_int64
    lib.axon_stop_nrt_profile.argtypes = [ctypes.c_char_p]
    lib.axon_stop_nrt_profile.restype = ctypes.c_int64

    @contextlib.contextmanager
    def _hook(output_dir, device_ids):
        import jax
        jax.devices()
        if device_ids:
            ids = (ctypes.c_int64 * len(device_ids))(*device_ids)
            rc = lib.axon_start_nrt_profile(ids, len(device_ids))
        else:
            rc = lib.axon_start_nrt_profile(None, 0)
        if rc != 0:
            raise RuntimeError(f"axon_start_nrt_profile rc={rc}")
        try:
            yield
        finally:
            n = lib.axon_stop_nrt_profile(str(output_dir).encode())
            print(f"ntff profile: {n} file(s) written to {output_dir}")

    mod = types.ModuleType("antenv.axon_hooks")
    mod.get_axon_ntff_profile_hook = lambda: _hook
    mod.set_axon_ntff_profile_hook = lambda h: None
    sys.modules["antenv.axon_hooks"] = mod


_install_ntff_hook()


# ----------------------------------------------------------------------------
# BIR post-processing: this container's walrus build rejects instructions with
# more than one sync wait; split extra waits onto preceding NoOps.
def _split_waits(bir_bytes, maxw=1):
    import json
    d = json.loads(bir_bytes)
    counter = [0]
    for fn in d["functions"]:
        for blk in fn["blocks"]:
            out = []
            for ins in blk["instructions"]:
                si = ins.get("sync_info")
                waits = (si or {}).get("on_wait") or []
                if len(waits) > maxw:
                    excess = waits[:-maxw]
                    ins["sync_info"]["on_wait"] = waits[-maxw:]
                    for i in range(0, len(excess), maxw):
                        counter[0] += 1
                        nop = {
                            "engine": ins["engine"],
                            "ins": [],
                            "outs": [],
                            "name": f"I-waitsplit-{counter[0]}",
                            "opcode": "NoOp",
                            "sync_info": {"on_update": [],
                                          "on_wait": excess[i:i + maxw]},
                        }
                        if "debug" in ins:
                            nop["debug"] = ins["debug"]
                        out.append(nop)
                out.append(ins)
            blk["instructions"] = out
    return json.dumps(d).encode()


# ----------------------------------------------------------------------------
# host-side positional embedding table (pure function of N, DM)
_POS_CACHE = {}


def _pos_embed():
    if "emb" in _POS_CACHE:
        return _POS_CACHE["emb"]
    n, fs = N, DM
    nb = fs // 6  # 256
    dist = np.arange(-n + 1, n, dtype=np.float64)
    adist = np.abs(dist)[:, None]

    max_range = math.log(n) / math.log(2.0)
    half_life = 2.0 ** np.linspace(3.0, max_range, nb)
    exp_feat = np.exp(-math.log(2.0) / half_life[None, :] * adist)

    with np.errstate(over="ignore"):
        center_widths = 2.0 ** np.arange(1, nb + 1, dtype=np.float64) - 1.0
    cmask_feat = (center_widths[None, :] > adist).astype(np.float64)

    stddev = n / (2.0 * nb)
    start_mean = n / nb
    mean = np.linspace(start_mean, float(n), nb)[None, :]
    conc = (mean / stddev) ** 2
    rate = mean / stddev ** 2
    with np.errstate(divide="ignore", invalid="ignore"):
        log_unnorm = (conc - 1.0) * np.log(adist) - rate * adist
    lgamma = np.vectorize(math.lgamma)
    log_norm = lgamma(conc) - conc * np.log(rate)
    with np.errstate(invalid="ignore"):
        prob = np.exp(log_unnorm - log_norm) + 1e-08
    prob = np.nan_to_num(prob, nan=1e-08)  # adist=0: 0*inf -> use limit 0, then +eps
    # recompute the adist == 0 row exactly: log_unnorm = -inf -> exp -> 0
    zrow = np.where(adist[:, 0] == 0)[0]
    prob[zrow, :] = 1e-08
    gamma_feat = prob / prob.max(axis=-1, keepdims=True)

    emb = np.concatenate([exp_feat, cmask_feat, gamma_feat], axis=-1)
    emb = np.concatenate([emb, np.sign(dist)[:, None] * emb], axis=-1)
    _POS_CACHE["emb"] = emb  # (4095, 1536) float64
    return emb


# ----------------------------------------------------------------------------
# device graph (identical for all cores; per-core data differs)
_GRAPH_CACHE = {}


def _build_graph():
    if "nc" in _GRAPH_CACHE:
        return _GRAPH_CACHE["nc"]
    debug = bool(os.environ.get("KERNEL_DEBUG"))
    nc = bass.Bass()

    xT = nc.declare_dram_parameter("xT", [DM, N], FP16, isOutput=False)
    wq = nc.declare_dram_parameter("wq", [DM, HD], FP16, isOutput=False)
    wkv = nc.declare_dram_parameter("wkv", [DM, 2 * HD], FP16, isOutput=False)
    ptab = nc.declare_dram_parameter("ptab", [HD, 2 * N], FP16, isOutput=False)
    cbias = nc.declare_dram_parameter("cbias", [HD, 1], F32, isOutput=False)
    pbias = nc.declare_dram_parameter("pbias", [HD, 1], F32, isOutput=False)
    wout = nc.declare_dram_parameter("wout", [INNER, 192], BF16, isOutput=False)
    bout = nc.declare_dram_parameter("bout", [1, 192], F32, isOutput=False)
    out_ext = nc.declare_dram_parameter("out", [N, 192], F32, isOutput=True)
    if debug:
        dbg_an = nc.declare_dram_parameter("dbg_an", [256, N], BF16, isOutput=True)
        dbg_og = nc.declare_dram_parameter("dbg_og", [HD, N], BF16, isOutput=True)
        dbg_sh = nc.declare_dram_parameter("dbg_sh", [256, N], FP16, isOutput=True)
        dbg_at = nc.declare_dram_parameter("dbg_at", [128, 2 * N], BF16, isOutput=True)
        dbg_rs = nc.declare_dram_parameter("dbg_rs", [HD, N], F32, isOutput=True)
    debug3 = bool(os.environ.get("KERNEL_DEBUG3"))
    if debug3:
        dbg3_og = nc.declare_dram_parameter("dbg3_og", [HD, 512], BF16, isOutput=True)
        dbg3_rs = nc.declare_dram_parameter("dbg3_rs", [HD, 512], F32, isOutput=True)
        dbg3_at = nc.declare_dram_parameter("dbg3_at", [128, 2 * N], BF16, isOutput=True)
        dbg3_an = nc.declare_dram_parameter("dbg3_an", [256, N], BF16, isOutput=True)
        dbg3_of = nc.declare_dram_parameter("dbg3_of", [512, 512], BF16, isOutput=True)

    # internal DRAM
    t_dram = [nc.dram_tensor(f"tscratch{a}", [128, TSTRIDE], FP16) for a in range(QT)]
    rs_dram = [nc.dram_tensor(f"rsrow{g}", [1, 512], F32) for g in range(4)]
    warm_in = nc.dram_tensor("warm_in", [1, 16], F32)
    warm_out = nc.dram_tensor("warm_out", [NCORES, 16], F32, addr_space="Shared")
    oag_in = [nc.dram_tensor(f"oag_in{g}", [HD, 512], BF16) for g in range(4)]
    oag_out = [nc.dram_tensor(f"oag_out{g}", [NCORES * HD, 512], BF16,
                              addr_space="Shared") for g in range(4)]

    groups = [list(range(NCORES))]
    Act = mybir.ActivationFunctionType

    with TileContext(nc) as tc:
        with contextlib.ExitStack() as ctx:
            persist = ctx.enter_context(tc.tile_pool(name="persist", bufs=1))
            work = ctx.enter_context(tc.tile_pool(name="work", bufs=2))

            # ---------------- phase 1: projections ----------------
            qcT = persist.tile([HD, N], FP16, tag="qcT")
            qpT = persist.tile([HD, N], FP16, tag="qpT")
            kvT = persist.tile([128, N], FP16, tag="kvT")   # k rows 0:64, v rows 64:128
            PT = persist.tile([HD, 2 * N], FP16, tag="PT")
            # vsb[k]: [seq 128, hd 64 | ones 1] -> row sums for free in attn*V
            vsb = [persist.tile([128, HD + 1], BF16, tag=f"v{k}", name=f"v{k}")
                   for k in range(QT)]
            ident = persist.tile([128, 128], FP16, tag="ident")
            make_identity(nc, ident)

            cb_sb = persist.tile([HD, 1], F32, tag="cb")
            pb_sb = persist.tile([HD, 1], F32, tag="pb")

            with contextlib.ExitStack() as ph1:
                wpool = ph1.enter_context(tc.tile_pool(name="wpool", bufs=1))
                xstream = ph1.enter_context(tc.tile_pool(name="xstream", bufs=2))
                ppsum = ph1.enter_context(
                    tc.tile_pool(name="ppsum", bufs=1, space="PSUM"))

                # first-tile operands first so PE starts ASAP
                wq_sb, wkv_sb, xts = [], [], []
                for f in range(12):
                    t = wpool.tile([128, HD], FP16, tag=f"wq{f}", name=f"wq{f}")
                    wq_sb.append(t)
                    t = wpool.tile([128, 2 * HD], FP16, tag=f"wkv{f}", name=f"wkv{f}")
                    wkv_sb.append(t)
                xt0 = xstream.tile([128, N], FP16, tag="xt", name="xt0")
                nc.sync.dma_start(out=xt0, in_=xT[0:128, :])
                nc.scalar.dma_start(out=wq_sb[0], in_=wq[0:128, :])
                nc.scalar.dma_start(out=wkv_sb[0], in_=wkv[0:128, :])
                nc.sync.dma_start(out=cb_sb, in_=cbias[:, :])
                nc.sync.dma_start(out=pb_sb, in_=pbias[:, :])
                # warmup collective: absorb CC bootstrap barrier early
                nc.gpsimd.collective_compute(
                    "AllGather", mybir.AluOpType.bypass, replica_groups=groups,
                    ins=[warm_in.ap().opt()], outs=[warm_out.ap().opt()])
                for f in range(1, 12):
                    eng = nc.sync if f % 2 == 0 else nc.scalar
                    eng.dma_start(out=wq_sb[f], in_=wq[128 * f:128 * (f + 1), :])
                    eng = nc.scalar if f % 2 == 0 else nc.sync
                    eng.dma_start(out=wkv_sb[f], in_=wkv[128 * f:128 * (f + 1), :])
                nc.gpsimd.dma_start(out=PT, in_=ptab[:, :])

                # single-pass projections: 8 accumulating psum banks
                q_ps = [ppsum.tile([HD, 512], F32, tag="pq", bufs=4,
                                   name=f"qps{i}") for i in range(4)]
                kv_ps = [ppsum.tile([128, 512], F32, tag="pkv", bufs=4,
                                    name=f"kvps{i}") for i in range(4)]
                for f in range(12):
                    if f == 0:
                        xt = xt0
                    else:
                        xt = xstream.tile([128, N], FP16, tag="xt")
                        eng = nc.sync if f % 2 == 0 else nc.scalar
                        eng.dma_start(out=xt, in_=xT[128 * f:128 * (f + 1), :])
                    for i in range(4):
                        nc.tensor.matmul(q_ps[i], wq_sb[f],
                                         xt[:, 512 * i:512 * (i + 1)],
                                         start=(f == 0), stop=(f == 11))
                    for i in range(4):
                        nc.tensor.matmul(kv_ps[i], wkv_sb[f],
                                         xt[:, 512 * i:512 * (i + 1)],
                                         start=(f == 0), stop=(f == 11))
                wout_sb = []
                for r in range(4):
                    t = persist.tile([128, 192], BF16, tag=f"wo{r}", name=f"wo{r}")
                    nc.gpsimd.dma_start(out=t, in_=wout[128 * r:128 * (r + 1), :])
                    wout_sb.append(t)
                bout_sb = persist.tile([128, 192], F32, tag="bo")
                bout_bc = bass.AP(tensor=bout.ap().tensor, offset=0,
                                  ap=[[0, 128], [1, 192]])
                nc.gpsimd.dma_start(out=bout_sb, in_=bout_bc)
                # copy-out: qpT first (unblocks T-matmul pipeline), then qcT/kv
                for i in range(4):
                    nc.scalar.activation(qpT[:, 512 * i:512 * (i + 1)], q_ps[i],
                                         Act.Identity, bias=pb_sb)
                for i in range(4):
                    nc.scalar.activation(qcT[:, 512 * i:512 * (i + 1)], q_ps[i],
                                         Act.Identity, bias=cb_sb)
                    nc.vector.tensor_copy(kvT[:, 512 * i:512 * (i + 1)], kv_ps[i])
                # ones column helper: f(0*x + 1) = 1 via ACT immediate
                onescol = persist.tile([128, 1], BF16, tag="ones")
                nc.scalar.activation(onescol, bout_sb[:, 0:1],
                                     Act.Identity, scale=0.0, bias=1.0)
                # v transpose to seq-major bf16 (+ ones column for row sums)
                for k in range(QT):
                    tp = ppsum.tile([128, HD], FP16, tag="pq", bufs=4)
                    nc.tensor.transpose(tp, kvT[HD:128, 128 * k:128 * (k + 1)],
                                        ident[HD:128, HD:128])
                    nc.vector.tensor_copy(vsb[k][:, 0:HD], tp)
                    nc.vector.tensor_copy(vsb[k][:, HD:HD + 1], onescol)

            # ---------------- phase 2: pipelined scores/softmax/av/AG/fin ---
            psum = ctx.enter_context(tc.tile_pool(name="psum", bufs=1, space="PSUM"))

            def produce(a):
                """T matmuls -> fp16 tsb -> DRAM write -> sheared read."""
                w0 = 1920 - 128 * a
                tsb = work.tile([128, WIN], FP16, tag="tsb", bufs=3,
                                name=f"tsb{a}")
                for ci, (off, w) in enumerate(CHUNKS):
                    tp = psum.tile([128, 512], F32, tag="T", bufs=2,
                                   name=f"tp{a}_{off}")
                    nc.tensor.matmul(tp[:, :w], qpT[:, 128 * a:128 * (a + 1)],
                                     PT[:, w0 + off:w0 + off + w],
                                     start=True, stop=True)
                    if ci in (1, 3):
                        nc.vector.tensor_copy(tsb[:, off:off + w], tp[:, :w])
                    else:
                        nc.scalar.copy(tsb[:, off:off + w], tp[:, :w])
                nc.gpsimd.dma_start(out=t_dram[a][:, 0:WIN], in_=tsb[:, 0:WIN])
                shear = work.tile([128, N], FP16, tag="shear", bufs=3,
                                  name=f"shear{a}")
                src = bass.AP(tensor=t_dram[a].ap().tensor, offset=127,
                              ap=[[TSTRIDE - 1, 128], [1, N]])
                nc.scalar.dma_start(out=shear, in_=src)
                return shear

            an = [persist.tile([128, N], BF16, tag=f"an{a}", name=f"an{a}")
                  for a in range(QT)]

            def consume(a, shear):
                """scores = qk (psum) + shear (DVE), exp (ACT) -> an[a]."""
                for j in range(4):
                    sp = psum.tile([128, 512], F32, tag="sc", bufs=3,
                                   name=f"sp{a}_{j}")
                    nc.tensor.matmul(sp, qcT[:, 128 * a:128 * (a + 1)],
                                     kvT[0:HD, 512 * j:512 * (j + 1)],
                                     start=True, stop=True)
                    nc.vector.tensor_add(sp, sp,
                                         shear[:, 512 * j:512 * (j + 1)])
                    nc.scalar.activation(an[a][:, 512 * j:512 * (j + 1)], sp,
                                         Act.Exp)
                if debug and a in (0, 6):
                    r = 0 if a == 0 else 128
                    nc.gpsimd.dma_start(out=dbg_an[r:r + 128, :], in_=an[a])
                    nc.gpsimd.dma_start(out=dbg_sh[r:r + 128, :], in_=shear)

            def at3(a):
                g, b = a // 4, a % 4
                if b == 0:
                    atG[g] = work.tile([128, 4, QT, 128], BF16, tag="atG",
                                       bufs=4, name=f"atG{g}")
                nc.sync.dma_start_transpose(atG[g][:, b, :, :], an[a])

            def av_group(g):
                """attn*V for 4 q-tiles; row 64 of otp = row sums (ones col)."""
                otp = psum.tile([HD + 1, 512], F32, tag="av", bufs=2,
                                name=f"otp{g}")
                for k in range(QT):
                    nc.tensor.matmul(otp, vsb[k], atG[g][:, :, k, :],
                                     start=(k == 0), stop=(k == QT - 1))
                rrow = work.tile([1, 512], F32, tag="rrow", bufs=2,
                                 name=f"rrow{g}")
                nc.scalar.copy(rrow, otp[HD:HD + 1, :])
                nc.sync.dma_start(out=rs_dram[g][:, :], in_=rrow)
                rsb = work.tile([HD, 512], F32, tag="rsb", bufs=2, name=f"rsb{g}")
                rs_bc = bass.AP(tensor=rs_dram[g].ap().tensor, offset=0,
                                ap=[[0, HD], [1, 512]])
                nc.scalar.dma_start(out=rsb, in_=rs_bc)
                nc.vector.reciprocal(rsb, rsb)
                og = work.tile([HD, 512], BF16, tag="og", bufs=4, name=f"og{g}")
                nc.vector.tensor_mul(og, otp[0:HD, :], rsb)
                if debug3 and g == 3:
                    dbg3_refs["og"] = og
                    dbg3_refs["rsb"] = rsb
                if debug:
                    nc.gpsimd.dma_start(out=dbg_og[:, 512 * g:512 * (g + 1)],
                                        in_=og)
                    nc.gpsimd.dma_start(out=dbg_rs[:, 512 * g:512 * (g + 1)],
                                        in_=rsb)
                    if g == 1:
                        nc.gpsimd.dma_start(out=dbg_at[:, 0:N],
                                            in_=atG[g][:, 2, :, :])
                        nc.gpsimd.dma_start(out=dbg_at[:, N:2 * N],
                                            in_=atG[g][:, 3, :, :])
                nc.scalar.dma_start(out=oag_in[g][:, :], in_=og)
                nc.gpsimd.collective_compute(
                    "AllGather", mybir.AluOpType.bypass, replica_groups=groups,
                    ins=[oag_in[g].ap().opt()], outs=[oag_out[g].ap().opt()])

            def fin(g):
                ofull = []
                for r in range(4):
                    t = work.tile([128, 512], BF16, tag="of", bufs=4,
                                  name=f"of{g}{r}")
                    eng = nc.sync if r % 2 == 0 else nc.scalar
                    eng.dma_start(out=t, in_=oag_out[g][128 * r:128 * (r + 1), :])
                    ofull.append(t)
                if debug3 and g == 3:
                    dbg3_refs["of"] = ofull
                for b in range(4):
                    fp = psum.tile([128, 192], F32, tag="fin", bufs=1,
                                   name=f"fp{g}_{b}")
                    for r in range(4):
                        nc.tensor.matmul(fp, ofull[r][:, 128 * b:128 * (b + 1)],
                                         wout_sb[r], start=(r == 0), stop=(r == 3))
                    ob = work.tile([128, 192], F32, tag="ob", bufs=2,
                                   name=f"ob{g}_{b}")
                    nc.vector.tensor_add(ob, fp, bout_sb)
                    m = 4 * g + b
                    eng = nc.sync if b % 2 == 0 else nc.scalar
                    eng.dma_start(out=out_ext[128 * m:128 * (m + 1), :], in_=ob)

            atG = {}
            dbg3_refs = {}

            LOOK = 3
            shears = {}
            for a in range(LOOK):
                shears[a] = produce(a)
            for a in range(QT):
                if a + LOOK < QT:
                    shears[a + LOOK] = produce(a + LOOK)
                consume(a, shears.pop(a))
                at3(a)
                if a % 4 == 3:
                    av_group(a // 4)
                if a == 13:
                    fin(0)
            fin(1)
            fin(2)
            fin(3)
            if debug3:
                nc.gpsimd.dma_start(out=dbg3_og[:, :], in_=dbg3_refs["og"])
                nc.gpsimd.dma_start(out=dbg3_rs[:, :], in_=dbg3_refs["rsb"])
                nc.gpsimd.dma_start(out=dbg3_at[:, 0:N], in_=atG[3][:, 2, :, :])
                nc.gpsimd.dma_start(out=dbg3_at[:, N:2 * N],
                                    in_=atG[3][:, 3, :, :])
                nc.gpsimd.dma_start(out=dbg3_an[0:128, :], in_=an[14])
                nc.gpsimd.dma_start(out=dbg3_an[128:256, :], in_=an[15])
                for r in range(4):
                    nc.gpsimd.dma_start(out=dbg3_of[128 * r:128 * (r + 1), :],
                                        in_=dbg3_refs["of"][r])

    # wait-split post-processing hook
    orig = nc.to_json_bytes
    nc.to_json_bytes = lambda: _split_waits(orig())
    _GRAPH_CACHE["nc"] = nc
    return nc


# ----------------------------------------------------------------------------
def _prep_inputs(x, Wq, Wk, Wv, content_bias, pos_bias, Wp_w, Wp_b, Wout_w, Wout_b):
    x = np.ascontiguousarray(np.asarray(x, dtype=np.float32))
    Wq = np.asarray(Wq, np.float32); Wk = np.asarray(Wk, np.float32)
    Wv = np.asarray(Wv, np.float32)
    content_bias = np.asarray(content_bias, np.float32)
    pos_bias = np.asarray(pos_bias, np.float32)
    Wp_w = np.asarray(Wp_w, np.float32); Wp_b = np.asarray(Wp_b, np.float32)
    Wout_w = np.asarray(Wout_w, np.float32); Wout_b = np.asarray(Wout_b, np.float32)

    scale = HD ** -0.5
    xT = np.ascontiguousarray(x[0].T)                    # (1536, 2048)
    emb = _pos_embed()                                   # (4095, 1536) f64
    wp_sum = Wp_w.reshape(DM, H, HD).sum(axis=1)         # (1536, 64)
    wp_b_sum = Wp_b.reshape(H, HD).sum(axis=0)           # (64,)
    # positional table: P = emb @ wp_sum + b  (weight preprocessing)
    P = emb @ wp_sum.astype(np.float64) + wp_b_sum.astype(np.float64)
    PTh = np.zeros((HD, 2 * N), np.float16)
    PTh[:, :2 * N - 1] = P.T.astype(np.float16)          # (64, 4096)
    xT16 = xT.astype(np.float16)

    in_maps = []
    for c in range(NCORES):
        sl = slice(HD * c, HD * (c + 1))
        in_maps.append({
            "xT": xT16,
            "wq": np.ascontiguousarray(Wq[:, sl] * scale).astype(np.float16),
            "wkv": np.ascontiguousarray(
                np.concatenate([Wk[:, sl], Wv[:, sl]], axis=1)).astype(np.float16),
            "ptab": PTh,
            "cbias": np.ascontiguousarray(content_bias[c, 0, :, None]),
            "pbias": np.ascontiguousarray(pos_bias[c, 0, :, None]),
            "wout": np.ascontiguousarray(
                Wout_w[:, 192 * c:192 * (c + 1)]).astype(ml_dtypes.bfloat16),
            "bout": np.ascontiguousarray(Wout_b[None, 192 * c:192 * (c + 1)]),
        })
    return in_maps


def kernel(x, Wq, Wk, Wv, content_bias, pos_bias, Wp_w, Wp_b, Wout_w, Wout_b):
    global _LAST_RESULT
    in_maps = _prep_inputs(x, Wq, Wk, Wv, content_bias, pos_bias,
                           Wp_w, Wp_b, Wout_w, Wout_b)
    nc = _build_graph()
    trace = bool(os.environ.get("KERNEL_TRACE"))
    res = run_bass_kernel_spmd(nc, in_maps, core_ids=list(range(NCORES)),
                               trace=trace, trace_cores=[0] if trace else None)
    _LAST_RESULT = res
    out = np.concatenate([res.results[c]["out"] for c in range(NCORES)], axis=1)
    return out[None].astype(np.float32)
